# revision 18
# baseline (speedup 1.0000x reference)
"""Correlation cost-volume kernel (max_displacement=4) for 8 Trainium2 cores.

Problem: in1, in2: [B=8, C=256, H=128, W=128] f32.
out[b, dy*9+dx, h, w] = sum_c in1[b,c,h,w] * pad(in2)[b, c, h+dy, w+dx]
(pad = 4 zeros on each spatial side), output [8, 81, 128, 128] f32.

Strategy (data-parallel, one batch sample per core):
  2D-tiled gram.  Each matmul tile covers an 8h x 16w block of in1 pixels
  (M = 128 PSUM partitions = pixels) against its 16 x 24 halo region of
  padded in2 (N = 384 columns, contracting C = 256 as two K=128 tiles
  accumulated in PSUM).  Every (pixel, displacement) pair the band needs is
  a (partition, column) entry of that [128 x 384] tile-gram, so the device
  streams 2x384 columns per 128 pixels -- 3.2x less TensorE work and 3.2x
  less output DMA than the per-row full-gram formulation.  The band entry
  for pixel (mh,mw) sits at column (mh+dy)*24 + (mw+dx): a per-partition
  (sheared) offset that no engine or DMA access pattern can express, so the
  device emits the full tile-grams (bf16, 4 tiles batched per DMA) and the
  host slices the 81-entry band per pixel with numpy stride tricks.
"""

import ml_dtypes
import numpy as np

import concourse.bass as bass
import concourse.bacc as bacc
import concourse.mybir as mybir
from concourse.bass_utils import run_bass_kernel_spmd
from concourse.tile import TileContext, add_dep_helper

B, C, H, W = 8, 256, 128, 128
D = 4
ND = 2 * D + 1  # 9 displacements per axis
HP = H + 2 * D  # 136 padded rows
WP = W + 2 * D  # 136 padded cols
KT = C // 128  # 2 contraction tiles
TH, TW = 8, 16  # matmul tile = 8h x 16w pixels (128 = one PSUM partition dim)
NHT, NWT = H // TH, W // TW  # 16 x 8 tiles
RH, RW = TH + 2 * D, TW + 2 * D  # 16 x 24 halo region, N = 384
NR = RH * RW  # 384 gram columns per tile

OUT_DT = mybir.dt.bfloat16
_OUT_NP = ml_dtypes.bfloat16

_CACHED_NC = None


def _build_nc():
    bf16 = mybir.dt.bfloat16

    nc = bacc.Bacc()
    # in1 as [c][ht][kt][wt][m=mh*16+mw]; in2 zero-padded as [c][kt][hp][wp]
    in1_t = nc.declare_dram_parameter("in1_t", [128, NHT, KT, NWT, 128], bf16, isOutput=False)
    in2_p = nc.declare_dram_parameter("in2_p", [128, KT, HP, WP], bf16, isOutput=False)
    # tile-grams: [ht-pair][m][i][wt][n]; partition m writes one 12KB run
    out_g = nc.declare_dram_parameter(
        "out_g", [NHT // 2, 128, 2, NWT, NR], OUT_DT, isOutput=True
    )

    with TileContext(nc) as tc:
        with (
            tc.tile_pool(name="bpool", bufs=1) as bpool,
            tc.tile_pool(name="apool", bufs=1) as apool,
            # deep output staging: 64 tile-grams can sit in SBUF, decoupling
            # compute from the output drain, which is gated to start only
            # after the input stream finishes (HBM reads pipeline at ~424GB/s
            # on one ring, but each HBM *write* DMA pays a ~2us completion
            # receipt, so concurrent reads+writes degrade both)
            tc.tile_pool(name="spool", bufs=4) as spool,
            tc.tile_pool(name="psum", bufs=6, space="PSUM") as ppool,
        ):
            # whole padded in2 sample resident in SBUF (74KB/partition),
            # loaded in 16-row chunks so early tiles can start before the
            # full 9.5MB lands (subtile deps give matmuls per-chunk waits)
            b_s = bpool.tile([128, KT, HP, WP], bf16)
            # whole in1 sample resident (64KB/partition), 2-ht chunks (1MB DMAs)
            a_s = apool.tile([128, NHT, KT, NWT, 128], bf16)

            def load_b(k):  # 16-row chunk k of padded in2
                r0 = 16 * k
                nr = min(16, HP - r0)
                return nc.sync.dma_start(
                    out=b_s[:, :, r0 : r0 + nr], in_=in2_p[:, :, r0 : r0 + nr]
                )

            def load_a(t):  # 2-ht chunk t of in1
                return nc.sync.dma_start(
                    out=a_s[:, 2 * t : 2 * t + 2], in_=in1_t[:, 2 * t : 2 * t + 2]
                )

            # all input DMAs issued up front (before any matmul reads, so no
            # conservative WAR hazards throttle the stream)
            last_in = None
            for k in range(9):
                last_in = load_b(k)
                if k < 8:
                    load_a(k)

            st = None
            for ht in range(NHT):
                h0 = TH * ht
                for wt in range(NWT):
                    w0 = TW * wt
                    idx = ht * NWT + wt
                    ps = ppool.tile([128, NR], mybir.dt.float32, name="ps", tag="ps")
                    for kt in range(KT):
                        nc.tensor.matmul(
                            ps,
                            a_s[:, ht, kt, wt, :],
                            b_s[:, kt, h0 : h0 + RH, w0 : w0 + RW],
                            start=(kt == 0),
                            stop=(kt == KT - 1),
                        )
                    if ht % 2 == 0 and wt == 0:
                        st = spool.tile([128, 2, NWT, NR], OUT_DT)
                    if idx % 2 == 0:
                        nc.vector.tensor_copy(st[:, ht % 2, wt, :], ps)
                    else:
                        nc.scalar.copy(st[:, ht % 2, wt, :], ps)
                    if ht % 2 == 1 and wt == NWT - 1:
                        # drain alternates between the two non-sync descriptor
                        # generators (ACT-HWDGE ring / SWDGE) so the ~2us HBM
                        # write-completion receipts of consecutive DMAs overlap
                        eng = nc.scalar if (ht // 2) % 2 == 0 else nc.gpsimd
                        od = eng.dma_start(out=out_g[ht // 2], in_=st)
                        # start the drain only once the input stream is done:
                        # mixed-direction HBM traffic degrades both streams
                        add_dep_helper(od.ins, last_in.ins, reason="drain after inputs")

    # bacc passes (move_matmul_waits_to_ldweights / generate_event_semaphores)
    # enforce the 1-wait-per-instruction HW constraint before serialization.
    nc.compile()
    return nc


def _get_nc():
    global _CACHED_NC
    if _CACHED_NC is None:
        _CACHED_NC = _build_nc()
    return _CACHED_NC


def _make_in_maps(in1: np.ndarray, in2: np.ndarray):
    in_maps = []
    for b in range(B):
        # [C,H,W] -> [c(128), ht, kt, wt, mh*16+mw]
        a = (
            in1[b]
            .astype(ml_dtypes.bfloat16)
            .reshape(KT, 128, NHT, TH, NWT, TW)
            .transpose(1, 2, 0, 4, 3, 5)
            .reshape(128, NHT, KT, NWT, 128)
        )
        p = np.zeros((KT, 128, HP, WP), ml_dtypes.bfloat16)
        p[:, :, D : D + H, D : D + W] = in2[b].astype(ml_dtypes.bfloat16).reshape(
            KT, 128, H, W
        )
        in_maps.append(
            {
                "in1_t": np.ascontiguousarray(a),
                "in2_p": np.ascontiguousarray(p.transpose(1, 0, 2, 3)),
            }
        )
    return in_maps


_IH = np.arange(TH)[:, None]
_JW = np.arange(TW)[None, :]


def _extract_band(g: np.ndarray) -> np.ndarray:
    """[NHT//2, 128, 2, NWT, NR] tile-grams -> [81, H, W] cost volume."""
    # -> [ht, wt, mh, mw, nh, nw]
    g2 = (
        g.reshape(NHT // 2, 128, 2, NWT, NR)
        .transpose(0, 2, 3, 1, 4)
        .reshape(NHT, NWT, TH, TW, RH, RW)
        .astype(np.float32)
    )
    # windows over (nh, nw): sw[.., a, b, dy, dx] = g2[.., a+dy, b+dx]
    sw = np.lib.stride_tricks.sliding_window_view(g2, (ND, ND), axis=(4, 5))
    band = sw[:, :, _IH, _JW, _IH, _JW]  # [ht, wt, mh, mw, dy, dx]
    # -> [dy, dx, ht, mh, wt, mw] -> [81, H, W]
    return np.ascontiguousarray(band.transpose(4, 5, 0, 2, 1, 3)).reshape(ND * ND, H, W)


def kernel(**inputs) -> np.ndarray:
    in1 = np.ascontiguousarray(np.asarray(inputs["in1"], dtype=np.float32))
    in2 = np.ascontiguousarray(np.asarray(inputs["in2"], dtype=np.float32))
    assert in1.shape == (B, C, H, W) and in2.shape == (B, C, H, W)

    nc = _get_nc()
    in_maps = _make_in_maps(in1, in2)
    res = run_bass_kernel_spmd(nc, in_maps, list(range(B)))

    outs = [_extract_band(np.asarray(res.results[b]["out_g"])) for b in range(B)]
    return np.stack(outs).astype(np.float32)


# revision 20
# speedup vs baseline: 1.0217x; 1.0217x over previous
"""Correlation cost-volume kernel (max_displacement=4) for 8 Trainium2 cores.

Problem: in1, in2: [B=8, C=256, H=128, W=128] f32.
out[b, dy*9+dx, h, w] = sum_c in1[b,c,h,w] * pad(in2)[b, c, h+dy, w+dx]
(pad = 4 zeros on each spatial side), output [8, 81, 128, 128] f32.

Strategy (data-parallel, one batch sample per core):
  2D-tiled gram.  Each matmul tile covers an 8h x 16w block of in1 pixels
  (M = 128 PSUM partitions = pixels) against its 16 x 24 halo region of
  padded in2 (N = 384 columns, contracting C = 256 as two K=128 tiles
  accumulated in PSUM).  Every (pixel, displacement) pair the band needs is
  a (partition, column) entry of that [128 x 384] tile-gram, so the device
  streams 2x384 columns per 128 pixels -- 3.2x less TensorE work and 3.2x
  less output DMA than the per-row full-gram formulation.  The band entry
  for pixel (mh,mw) sits at column (mh+dy)*24 + (mw+dx): a per-partition
  (sheared) offset that no engine or DMA access pattern can express, so the
  device emits the full tile-grams (bf16, 4 tiles batched per DMA) and the
  host slices the 81-entry band per pixel with numpy stride tricks.
"""

import ml_dtypes
import numpy as np

import concourse.bass as bass
import concourse.bacc as bacc
import concourse.mybir as mybir
from concourse.bass_utils import run_bass_kernel_spmd
from concourse.tile import TileContext, add_dep_helper

B, C, H, W = 8, 256, 128, 128
D = 4
ND = 2 * D + 1  # 9 displacements per axis
HP = H + 2 * D  # 136 padded rows
WP = W + 2 * D  # 136 padded cols
KT = C // 128  # 2 contraction tiles
TH, TW = 8, 16  # matmul tile = 8h x 16w pixels (128 = one PSUM partition dim)
NHT, NWT = H // TH, W // TW  # 16 x 8 tiles
RH, RW = TH + 2 * D, TW + 2 * D  # 16 x 24 halo region, N = 384
NR = RH * RW  # 384 gram columns per tile

OUT_DT = mybir.dt.bfloat16
_OUT_NP = ml_dtypes.bfloat16

_CACHED_NC = None


def _build_nc():
    bf16 = mybir.dt.bfloat16

    nc = bacc.Bacc()
    # in1 as [c][ht][kt][wt][m=mh*16+mw]; in2 zero-padded in w only, as
    # [c][kt][h(128)][wp] -- the 8 zero pad rows are memset on-chip
    in1_t = nc.declare_dram_parameter("in1_t", [128, NHT, KT, NWT, 128], bf16, isOutput=False)
    in2_p = nc.declare_dram_parameter("in2_p", [128, KT, H, WP], bf16, isOutput=False)
    # tile-grams: [ht-pair][m][i][wt][n]; partition m writes one 12KB run
    out_g = nc.declare_dram_parameter(
        "out_g", [NHT // 2, 128, 2, NWT, NR], OUT_DT, isOutput=True
    )

    with TileContext(nc) as tc:
        with (
            tc.tile_pool(name="bpool", bufs=1) as bpool,
            tc.tile_pool(name="apool", bufs=3) as apool,
            # the WHOLE output (8 ht-pair grams, 96KB/partition) stages in
            # SBUF, so compute never stalls on the drain.  The drain is gated
            # to start only after the input stream finishes (HBM reads
            # pipeline at ~424GB/s on one ring, but each HBM *write* DMA pays
            # a ~2us completion receipt, so mixing directions degrades both),
            # and alternates between the sync-HWDGE ring (FIFO after the
            # inputs anyway) and the otherwise-idle SWDGE ring so consecutive
            # write receipts overlap.  Queue discipline: scalar/vector carry
            # ONLY copies -- a waiting DMA parked on the scalar queue would
            # head-of-line block the copy pipeline (the v6 failure mode).
            tc.tile_pool(name="spool", bufs=8) as spool,
            tc.tile_pool(name="psum", bufs=6, space="PSUM") as ppool,
        ):
            # whole padded in2 sample resident in SBUF (74KB/partition),
            # loaded in 16-row chunks so early tiles can start before the
            # full 8.4MB lands (subtile deps give matmuls per-chunk waits)
            b_s = bpool.tile([128, KT, HP, WP], bf16)
            # zero the 4 pad rows top+bottom (pad cols ship from DRAM)
            nc.gpsimd.memset(b_s[:, :, 0:D, :], 0.0)
            nc.gpsimd.memset(b_s[:, :, D + H :, :], 0.0)

            def load_b(k):  # 16-row chunk k of the unpadded-row interior
                return nc.sync.dma_start(
                    out=b_s[:, :, D + 16 * k : D + 16 * k + 16],
                    in_=in2_p[:, :, 16 * k : 16 * k + 16],
                )

            # in1 rotates through 3 chunk buffers (2 ht rows each); its loads
            # ride the SWDGE ring so their buffer-reuse waits cannot stall
            # the free-running b-stream on the sync ring
            a_tiles = []

            def load_a(t):
                a_t = apool.tile([128, 2, KT, NWT, 128], bf16, tag="a")
                a_tiles.append(a_t)
                return nc.gpsimd.dma_start(
                    out=a_t, in_=in1_t[:, 2 * t : 2 * t + 2]
                )

            # all input DMAs issued up front (before any matmul reads, so no
            # conservative WAR hazards throttle the stream)
            last_in = None
            for k in range(8):
                last_in = load_b(k)
                load_a(k)

            st = None
            for ht in range(NHT):
                h0 = TH * ht
                for wt in range(NWT):
                    w0 = TW * wt
                    idx = ht * NWT + wt
                    ps = ppool.tile([128, NR], mybir.dt.float32, name="ps", tag="ps")
                    for kt in range(KT):
                        nc.tensor.matmul(
                            ps,
                            a_tiles[ht // 2][:, ht % 2, kt, wt, :],
                            b_s[:, kt, h0 : h0 + RH, w0 : w0 + RW],
                            start=(kt == 0),
                            stop=(kt == KT - 1),
                        )
                    if ht % 2 == 0 and wt == 0:
                        st = spool.tile([128, 2, NWT, NR], OUT_DT)
                    if idx % 2 == 0:
                        nc.vector.tensor_copy(st[:, ht % 2, wt, :], ps)
                    else:
                        nc.scalar.copy(st[:, ht % 2, wt, :], ps)
                    if ht % 2 == 1 and wt == NWT - 1:
                        eng = nc.sync if (ht // 2) % 2 == 0 else nc.gpsimd
                        od = eng.dma_start(out=out_g[ht // 2], in_=st)
                        add_dep_helper(od.ins, last_in.ins, reason="drain after inputs")

    # bacc passes (move_matmul_waits_to_ldweights / generate_event_semaphores)
    # enforce the 1-wait-per-instruction HW constraint before serialization.
    nc.compile()
    return nc


def _get_nc():
    global _CACHED_NC
    if _CACHED_NC is None:
        _CACHED_NC = _build_nc()
    return _CACHED_NC


def _make_in_maps(in1: np.ndarray, in2: np.ndarray):
    in_maps = []
    for b in range(B):
        # [C,H,W] -> [c(128), ht, kt, wt, mh*16+mw]
        a = (
            in1[b]
            .astype(ml_dtypes.bfloat16)
            .reshape(KT, 128, NHT, TH, NWT, TW)
            .transpose(1, 2, 0, 4, 3, 5)
            .reshape(128, NHT, KT, NWT, 128)
        )
        p = np.zeros((KT, 128, H, WP), ml_dtypes.bfloat16)
        p[:, :, :, D : D + W] = in2[b].astype(ml_dtypes.bfloat16).reshape(
            KT, 128, H, W
        )
        in_maps.append(
            {
                "in1_t": np.ascontiguousarray(a),
                "in2_p": np.ascontiguousarray(p.transpose(1, 0, 2, 3)),
            }
        )
    return in_maps


_IH = np.arange(TH)[:, None]
_JW = np.arange(TW)[None, :]


def _extract_band(g: np.ndarray) -> np.ndarray:
    """[NHT//2, 128, 2, NWT, NR] tile-grams -> [81, H, W] cost volume."""
    # -> [ht, wt, mh, mw, nh, nw]
    g2 = (
        g.reshape(NHT // 2, 128, 2, NWT, NR)
        .transpose(0, 2, 3, 1, 4)
        .reshape(NHT, NWT, TH, TW, RH, RW)
        .astype(np.float32)
    )
    # windows over (nh, nw): sw[.., a, b, dy, dx] = g2[.., a+dy, b+dx]
    sw = np.lib.stride_tricks.sliding_window_view(g2, (ND, ND), axis=(4, 5))
    band = sw[:, :, _IH, _JW, _IH, _JW]  # [ht, wt, mh, mw, dy, dx]
    # -> [dy, dx, ht, mh, wt, mw] -> [81, H, W]
    return np.ascontiguousarray(band.transpose(4, 5, 0, 2, 1, 3)).reshape(ND * ND, H, W)


def kernel(**inputs) -> np.ndarray:
    in1 = np.ascontiguousarray(np.asarray(inputs["in1"], dtype=np.float32))
    in2 = np.ascontiguousarray(np.asarray(inputs["in2"], dtype=np.float32))
    assert in1.shape == (B, C, H, W) and in2.shape == (B, C, H, W)

    nc = _get_nc()
    in_maps = _make_in_maps(in1, in2)
    res = run_bass_kernel_spmd(nc, in_maps, list(range(B)))

    outs = [_extract_band(np.asarray(res.results[b]["out_g"])) for b in range(B)]
    return np.stack(outs).astype(np.float32)


# revision 22
# speedup vs baseline: 1.0301x; 1.0083x over previous
"""Correlation cost-volume kernel (max_displacement=4) for 8 Trainium2 cores.

Problem: in1, in2: [B=8, C=256, H=128, W=128] f32.
out[b, dy*9+dx, h, w] = sum_c in1[b,c,h,w] * pad(in2)[b, c, h+dy, w+dx]
(pad = 4 zeros on each spatial side), output [8, 81, 128, 128] f32.

Strategy (data-parallel, one batch sample per core):
  2D-tiled gram.  Each matmul tile covers an 8h x 16w block of in1 pixels
  (M = 128 PSUM partitions = pixels) against its 16 x 24 halo region of
  padded in2 (N = 384 columns, contracting C = 256 as two K=128 tiles
  accumulated in PSUM).  Every (pixel, displacement) pair the band needs is
  a (partition, column) entry of that [128 x 384] tile-gram, so the device
  streams 2x384 columns per 128 pixels -- 3.2x less TensorE work and 3.2x
  less output DMA than the per-row full-gram formulation.  The band entry
  for pixel (mh,mw) sits at column (mh+dy)*24 + (mw+dx): a per-partition
  (sheared) offset that no engine or DMA access pattern can express, so the
  device emits the full tile-grams (bf16, 4 tiles batched per DMA) and the
  host slices the 81-entry band per pixel with numpy stride tricks.
"""

import ml_dtypes
import numpy as np

import concourse.bass as bass
import concourse.bacc as bacc
import concourse.mybir as mybir
from concourse.bass_utils import run_bass_kernel_spmd
from concourse.tile import TileContext, add_dep_helper

B, C, H, W = 8, 256, 128, 128
D = 4
ND = 2 * D + 1  # 9 displacements per axis
HP = H + 2 * D  # 136 padded rows
WP = W + 2 * D  # 136 padded cols
KT = C // 128  # 2 contraction tiles
TH, TW = 8, 16  # matmul tile = 8h x 16w pixels (128 = one PSUM partition dim)
NHT, NWT = H // TH, W // TW  # 16 x 8 tiles
RH, RW = TH + 2 * D, TW + 2 * D  # 16 x 24 halo region, N = 384
NR = RH * RW  # 384 gram columns per tile

OUT_DT = mybir.dt.bfloat16
_OUT_NP = ml_dtypes.bfloat16

_CACHED_NC = None


def _build_nc():
    bf16 = mybir.dt.bfloat16

    nc = bacc.Bacc()
    # in1 as [c][ht][kt][wt][m=mh*16+mw]; in2 zero-padded in w only, as
    # [c][kt][h(128)][wp] -- the 8 zero pad rows are memset on-chip
    in1_t = nc.declare_dram_parameter("in1_t", [128, NHT, KT, NWT, 128], bf16, isOutput=False)
    in2_p = nc.declare_dram_parameter("in2_p", [128, KT, H, WP], bf16, isOutput=False)
    # tile-grams: [ht-pair][m][i][wt][n]; partition m writes one 12KB run
    out_g = nc.declare_dram_parameter(
        "out_g", [NHT // 2, 128, 2, NWT, NR], OUT_DT, isOutput=True
    )

    with TileContext(nc) as tc:
        with (
            tc.tile_pool(name="bpool", bufs=1) as bpool,
            tc.tile_pool(name="apool", bufs=3) as apool,
            # the WHOLE output (8 ht-pair grams, 96KB/partition) stages in
            # SBUF, so compute never stalls on the drain.  The drain is gated
            # to start only after the input stream finishes (HBM reads
            # pipeline at ~424GB/s on one ring, but each HBM *write* DMA pays
            # a ~2us completion receipt, so mixing directions degrades both),
            # and alternates between the sync-HWDGE ring (FIFO after the
            # inputs anyway) and the otherwise-idle SWDGE ring so consecutive
            # write receipts overlap.  Queue discipline: scalar/vector carry
            # ONLY copies -- a waiting DMA parked on the scalar queue would
            # head-of-line block the copy pipeline (the v6 failure mode).
            tc.tile_pool(name="spool", bufs=8) as spool,
            tc.tile_pool(name="psum", bufs=6, space="PSUM") as ppool,
        ):
            # whole padded in2 sample resident in SBUF (74KB/partition),
            # loaded in 16-row chunks so early tiles can start before the
            # full 8.4MB lands (subtile deps give matmuls per-chunk waits)
            b_s = bpool.tile([128, KT, HP, WP], bf16)
            # zero the 4 pad rows top+bottom (pad cols ship from DRAM)
            nc.gpsimd.memset(b_s[:, :, 0:D, :], 0.0)
            nc.gpsimd.memset(b_s[:, :, D + H :, :], 0.0)

            def load_b(k):  # 16-row chunk k of the unpadded-row interior
                return nc.sync.dma_start(
                    out=b_s[:, :, D + 16 * k : D + 16 * k + 16],
                    in_=in2_p[:, :, 16 * k : 16 * k + 16],
                )

            # in1 rotates through 3 chunk buffers (2 ht rows each); its loads
            # ride the SWDGE ring so their buffer-reuse waits cannot stall
            # the free-running b-stream on the sync ring
            a_tiles = []

            def load_a(t):
                a_t = apool.tile([128, 2, KT, NWT, 128], bf16, tag="a")
                a_tiles.append(a_t)
                return nc.gpsimd.dma_start(
                    out=a_t, in_=in1_t[:, 2 * t : 2 * t + 2]
                )

            # all input DMAs issued up front (before any matmul reads, so no
            # conservative WAR hazards throttle the stream)
            last_b = last_a = None
            for k in range(8):
                last_b = load_b(k)
                last_a = load_a(k)

            st = None
            for ht in range(NHT):
                h0 = TH * ht
                for wt in range(NWT):
                    w0 = TW * wt
                    idx = ht * NWT + wt
                    ps = ppool.tile([128, NR], mybir.dt.float32, name="ps", tag="ps")
                    for kt in range(KT):
                        nc.tensor.matmul(
                            ps,
                            a_tiles[ht // 2][:, ht % 2, kt, wt, :],
                            b_s[:, kt, h0 : h0 + RH, w0 : w0 + RW],
                            start=(kt == 0),
                            stop=(kt == KT - 1),
                        )
                    if ht % 2 == 0 and wt == 0:
                        st = spool.tile([128, 2, NWT, NR], OUT_DT)
                    if idx % 2 == 0:
                        nc.vector.tensor_copy(st[:, ht % 2, wt, :], ps)
                    else:
                        nc.scalar.copy(st[:, ht % 2, wt, :], ps)
                    if ht % 2 == 1 and wt == NWT - 1:
                        eng = nc.sync if (ht // 2) % 2 == 0 else nc.gpsimd
                        od = eng.dma_start(out=out_g[ht // 2], in_=st)
                        # gate on BOTH input streams: without the a-gate the
                        # scheduler orders drain writes ahead of the last
                        # in1 chunk loads on the shared SWDGE ring, and the
                        # matmuls then stall ~20us behind 1.5MB writes
                        add_dep_helper(od.ins, last_b.ins, reason="drain after in2")
                        add_dep_helper(od.ins, last_a.ins, reason="drain after in1")

    # bacc passes (move_matmul_waits_to_ldweights / generate_event_semaphores)
    # enforce the 1-wait-per-instruction HW constraint before serialization.
    nc.compile()
    return nc


def _get_nc():
    global _CACHED_NC
    if _CACHED_NC is None:
        _CACHED_NC = _build_nc()
    return _CACHED_NC


def _make_in_maps(in1: np.ndarray, in2: np.ndarray):
    in_maps = []
    for b in range(B):
        # [C,H,W] -> [c(128), ht, kt, wt, mh*16+mw]
        a = (
            in1[b]
            .astype(ml_dtypes.bfloat16)
            .reshape(KT, 128, NHT, TH, NWT, TW)
            .transpose(1, 2, 0, 4, 3, 5)
            .reshape(128, NHT, KT, NWT, 128)
        )
        p = np.zeros((KT, 128, H, WP), ml_dtypes.bfloat16)
        p[:, :, :, D : D + W] = in2[b].astype(ml_dtypes.bfloat16).reshape(
            KT, 128, H, W
        )
        in_maps.append(
            {
                "in1_t": np.ascontiguousarray(a),
                "in2_p": np.ascontiguousarray(p.transpose(1, 0, 2, 3)),
            }
        )
    return in_maps


_IH = np.arange(TH)[:, None]
_JW = np.arange(TW)[None, :]


def _extract_band(g: np.ndarray) -> np.ndarray:
    """[NHT//2, 128, 2, NWT, NR] tile-grams -> [81, H, W] cost volume."""
    # -> [ht, wt, mh, mw, nh, nw]
    g2 = (
        g.reshape(NHT // 2, 128, 2, NWT, NR)
        .transpose(0, 2, 3, 1, 4)
        .reshape(NHT, NWT, TH, TW, RH, RW)
        .astype(np.float32)
    )
    # windows over (nh, nw): sw[.., a, b, dy, dx] = g2[.., a+dy, b+dx]
    sw = np.lib.stride_tricks.sliding_window_view(g2, (ND, ND), axis=(4, 5))
    band = sw[:, :, _IH, _JW, _IH, _JW]  # [ht, wt, mh, mw, dy, dx]
    # -> [dy, dx, ht, mh, wt, mw] -> [81, H, W]
    return np.ascontiguousarray(band.transpose(4, 5, 0, 2, 1, 3)).reshape(ND * ND, H, W)


def kernel(**inputs) -> np.ndarray:
    in1 = np.ascontiguousarray(np.asarray(inputs["in1"], dtype=np.float32))
    in2 = np.ascontiguousarray(np.asarray(inputs["in2"], dtype=np.float32))
    assert in1.shape == (B, C, H, W) and in2.shape == (B, C, H, W)

    nc = _get_nc()
    in_maps = _make_in_maps(in1, in2)
    res = run_bass_kernel_spmd(nc, in_maps, list(range(B)))

    outs = [_extract_band(np.asarray(res.results[b]["out_g"])) for b in range(B)]
    return np.stack(outs).astype(np.float32)


# revision 26
# speedup vs baseline: 1.0349x; 1.0046x over previous
"""Correlation cost-volume kernel (max_displacement=4) for 8 Trainium2 cores.

Problem: in1, in2: [B=8, C=256, H=128, W=128] f32.
out[b, dy*9+dx, h, w] = sum_c in1[b,c,h,w] * pad(in2)[b, c, h+dy, w+dx]
(pad = 4 zeros on each spatial side), output [8, 81, 128, 128] f32.

Strategy (data-parallel, one batch sample per core):
  2D-tiled gram.  Each matmul tile covers an 8h x 16w block of in1 pixels
  (M = 128 PSUM partitions = pixels) against its 16 x 24 halo region of
  padded in2 (N = 384 columns, contracting C = 256 as two K=128 tiles
  accumulated in PSUM).  Every (pixel, displacement) pair the band needs is
  a (partition, column) entry of that [128 x 384] tile-gram, so the device
  streams 2x384 columns per 128 pixels -- 3.2x less TensorE work and 3.2x
  less output DMA than the per-row full-gram formulation.  The band entry
  for pixel (mh,mw) sits at column (mh+dy)*24 + (mw+dx): a per-partition
  (sheared) offset that no engine or DMA access pattern can express, so the
  device emits the full tile-grams (bf16, 4 tiles batched per DMA) and the
  host slices the 81-entry band per pixel with numpy stride tricks.
"""

import ml_dtypes
import numpy as np

import concourse.bass as bass
import concourse.bacc as bacc
import concourse.mybir as mybir
from concourse.bass_utils import run_bass_kernel_spmd
from concourse.tile import TileContext, add_dep_helper

B, C, H, W = 8, 256, 128, 128
D = 4
ND = 2 * D + 1  # 9 displacements per axis
HP = H + 2 * D  # 136 padded rows
WP = W + 2 * D  # 136 padded cols
KT = C // 128  # 2 contraction tiles
TH, TW = 8, 16  # matmul tile = 8h x 16w pixels (128 = one PSUM partition dim)
NHT, NWT = H // TH, W // TW  # 16 x 8 tiles
RH, RW = TH + 2 * D, TW + 2 * D  # 16 x 24 halo region, N = 384
NR = RH * RW  # 384 gram columns per tile

OUT_DT = mybir.dt.bfloat16
_OUT_NP = ml_dtypes.bfloat16

_CACHED_NC = None


def _build_nc():
    bf16 = mybir.dt.bfloat16

    nc = bacc.Bacc()
    # in1 as [c][ht][kt][wt][m=mh*16+mw]; in2 zero-padded in w only, as
    # [c][kt][h(128)][wp] -- the 8 zero pad rows are memset on-chip
    in1_t = nc.declare_dram_parameter("in1_t", [128, NHT, KT, NWT, 128], bf16, isOutput=False)
    in2_p = nc.declare_dram_parameter("in2_p", [128, KT, H, WP], bf16, isOutput=False)
    # tile-grams: [ht][m][wt][n]; partition m writes one contiguous 6KB run
    out_g = nc.declare_dram_parameter(
        "out_g", [NHT, 128, NWT, NR], OUT_DT, isOutput=True
    )

    with TileContext(nc) as tc:
        with (
            tc.tile_pool(name="bpool", bufs=1) as bpool,
            tc.tile_pool(name="apool", bufs=1) as apool,
            # 8 ht rows of tile-grams (48KB/partition) stage in SBUF so
            # compute rarely stalls on the drain.  The drain is gated to
            # start only after the input stream finishes (HBM reads pipeline
            # at ~424GB/s on one ring, but each HBM *write* DMA pays a ~2us
            # completion receipt, so mixing directions degrades both), and
            # alternates between the sync-HWDGE ring (FIFO after the inputs
            # anyway) and the otherwise-idle SWDGE ring so consecutive write
            # receipts overlap.  Queue discipline: scalar/vector carry ONLY
            # copies, and ALL input loads ride the free-running sync ring --
            # a waiting DMA parked on the scalar queue head-of-line blocks
            # the copy pipeline, and input loads sharing a ring with gated
            # drain writes get starved behind them.
            tc.tile_pool(name="spool", bufs=8) as spool,
            tc.tile_pool(name="psum", bufs=6, space="PSUM") as ppool,
        ):
            # whole padded in2 sample resident in SBUF (72KB/partition),
            # loaded in 16-row chunks so early tiles can start before the
            # full 8.4MB lands (subtile deps give matmuls per-chunk waits)
            b_s = bpool.tile([128, KT, HP, WP], bf16)
            # zero the 4 pad rows top+bottom (pad cols ship from DRAM)
            nc.gpsimd.memset(b_s[:, :, 0:D, :], 0.0)
            nc.gpsimd.memset(b_s[:, :, D + H :, :], 0.0)
            # whole in1 sample resident (64KB/partition), 2-ht chunks (1MB)
            a_s = apool.tile([128, NHT, KT, NWT, 128], bf16)

            def load_b(k):  # 16-row chunk k of the unpadded-row interior
                return nc.sync.dma_start(
                    out=b_s[:, :, D + 16 * k : D + 16 * k + 16],
                    in_=in2_p[:, :, 16 * k : 16 * k + 16],
                )

            def load_a(t):  # 2-ht chunk t of in1
                return nc.sync.dma_start(
                    out=a_s[:, 2 * t : 2 * t + 2], in_=in1_t[:, 2 * t : 2 * t + 2]
                )

            # all input DMAs issued up front (before any matmul reads, so no
            # conservative WAR hazards throttle the stream)
            last_b = last_a = None
            for k in range(8):
                last_b = load_b(k)
                last_a = load_a(k)

            st = None
            for ht in range(NHT):
                h0 = TH * ht
                for wt in range(NWT):
                    w0 = TW * wt
                    idx = ht * NWT + wt
                    ps = ppool.tile([128, NR], mybir.dt.float32, name="ps", tag="ps")
                    for kt in range(KT):
                        nc.tensor.matmul(
                            ps,
                            a_s[:, ht, kt, wt, :],
                            b_s[:, kt, h0 : h0 + RH, w0 : w0 + RW],
                            start=(kt == 0),
                            stop=(kt == KT - 1),
                        )
                    if wt == 0:
                        st = spool.tile([128, NWT, NR], OUT_DT)
                    if idx % 2 == 0:
                        nc.vector.tensor_copy(st[:, wt, :], ps)
                    else:
                        nc.scalar.copy(st[:, wt, :], ps)
                    if wt == NWT - 1:
                        if ht % 2 == 0:
                            # sync ring: FIFO already orders this transfer
                            # after the whole input stream
                            nc.sync.dma_start(out=out_g[ht], in_=st)
                        else:
                            od = nc.gpsimd.dma_start(out=out_g[ht], in_=st)
                            add_dep_helper(od.ins, last_b.ins, reason="drain after in2")
                            add_dep_helper(od.ins, last_a.ins, reason="drain after in1")

    # bacc passes (move_matmul_waits_to_ldweights / generate_event_semaphores)
    # enforce the 1-wait-per-instruction HW constraint before serialization.
    nc.compile()
    return nc


def _get_nc():
    global _CACHED_NC
    if _CACHED_NC is None:
        _CACHED_NC = _build_nc()
    return _CACHED_NC


def _make_in_maps(in1: np.ndarray, in2: np.ndarray):
    in_maps = []
    for b in range(B):
        # [C,H,W] -> [c(128), ht, kt, wt, mh*16+mw]
        a = (
            in1[b]
            .astype(ml_dtypes.bfloat16)
            .reshape(KT, 128, NHT, TH, NWT, TW)
            .transpose(1, 2, 0, 4, 3, 5)
            .reshape(128, NHT, KT, NWT, 128)
        )
        p = np.zeros((KT, 128, H, WP), ml_dtypes.bfloat16)
        p[:, :, :, D : D + W] = in2[b].astype(ml_dtypes.bfloat16).reshape(
            KT, 128, H, W
        )
        in_maps.append(
            {
                "in1_t": np.ascontiguousarray(a),
                "in2_p": np.ascontiguousarray(p.transpose(1, 0, 2, 3)),
            }
        )
    return in_maps


_IH = np.arange(TH)[:, None]
_JW = np.arange(TW)[None, :]


def _extract_band(g: np.ndarray) -> np.ndarray:
    """[NHT, 128, NWT, NR] tile-grams -> [81, H, W] cost volume."""
    # -> [ht, wt, mh, mw, nh, nw]
    g2 = (
        g.reshape(NHT, 128, NWT, NR)
        .transpose(0, 2, 1, 3)
        .reshape(NHT, NWT, TH, TW, RH, RW)
        .astype(np.float32)
    )
    # windows over (nh, nw): sw[.., a, b, dy, dx] = g2[.., a+dy, b+dx]
    sw = np.lib.stride_tricks.sliding_window_view(g2, (ND, ND), axis=(4, 5))
    band = sw[:, :, _IH, _JW, _IH, _JW]  # [ht, wt, mh, mw, dy, dx]
    # -> [dy, dx, ht, mh, wt, mw] -> [81, H, W]
    return np.ascontiguousarray(band.transpose(4, 5, 0, 2, 1, 3)).reshape(ND * ND, H, W)


def kernel(**inputs) -> np.ndarray:
    in1 = np.ascontiguousarray(np.asarray(inputs["in1"], dtype=np.float32))
    in2 = np.ascontiguousarray(np.asarray(inputs["in2"], dtype=np.float32))
    assert in1.shape == (B, C, H, W) and in2.shape == (B, C, H, W)

    nc = _get_nc()
    in_maps = _make_in_maps(in1, in2)
    res = run_bass_kernel_spmd(nc, in_maps, list(range(B)))

    outs = [_extract_band(np.asarray(res.results[b]["out_g"])) for b in range(B)]
    return np.stack(outs).astype(np.float32)


# revision 28
# speedup vs baseline: 1.0695x; 1.0334x over previous
"""Correlation cost-volume kernel (max_displacement=4) for 8 Trainium2 cores.

Problem: in1, in2: [B=8, C=256, H=128, W=128] f32.
out[b, dy*9+dx, h, w] = sum_c in1[b,c,h,w] * pad(in2)[b, c, h+dy, w+dx]
(pad = 4 zeros on each spatial side), output [8, 81, 128, 128] f32.

Strategy (data-parallel, one batch sample per core):
  2D-tiled gram.  Each matmul tile covers an 8h x 16w block of in1 pixels
  (M = 128 PSUM partitions = pixels) against its 16 x 24 halo region of
  padded in2 (N = 384 columns, contracting C = 256 as two K=128 tiles
  accumulated in PSUM).  Every (pixel, displacement) pair the band needs is
  a (partition, column) entry of that [128 x 384] tile-gram, so the device
  streams 2x384 columns per 128 pixels -- 3.2x less TensorE work and 3.2x
  less output DMA than the per-row full-gram formulation.  The band entry
  for pixel (mh,mw) sits at column (mh+dy)*24 + (mw+dx): a per-partition
  (sheared) offset that no engine or DMA access pattern can express, so the
  device emits the full tile-grams (bf16, 4 tiles batched per DMA) and the
  host slices the 81-entry band per pixel with numpy stride tricks.
"""

import ml_dtypes
import numpy as np

import concourse.bass as bass
import concourse.bacc as bacc
import concourse.mybir as mybir
from concourse.bass_utils import run_bass_kernel_spmd
from concourse.tile import TileContext, add_dep_helper

B, C, H, W = 8, 256, 128, 128
D = 4
ND = 2 * D + 1  # 9 displacements per axis
HP = H + 2 * D  # 136 padded rows
WP = W + 2 * D  # 136 padded cols
KT = C // 128  # 2 contraction tiles
TH, TW = 8, 16  # matmul tile = 8h x 16w pixels (128 = one PSUM partition dim)
NHT, NWT = H // TH, W // TW  # 16 x 8 tiles
RH, RW = TH + 2 * D, TW + 2 * D  # 16 x 24 halo region, N = 384
NR = RH * RW  # 384 gram columns per tile

OUT_DT = mybir.dt.bfloat16
_OUT_NP = ml_dtypes.bfloat16

_CACHED_NC = None


def _build_nc():
    bf16 = mybir.dt.bfloat16

    nc = bacc.Bacc()
    # in1 as [c][ht][kt][wt][m=mh*16+mw]; in2 zero-padded in w only, as
    # [c][kt][h(128)][wp] -- the 8 zero pad rows are memset on-chip
    in1_t = nc.declare_dram_parameter("in1_t", [128, NHT, KT, NWT, 128], bf16, isOutput=False)
    in2_p = nc.declare_dram_parameter("in2_p", [128, KT, H, WP], bf16, isOutput=False)
    # tile-grams: [ht][m][wt][n]; partition m writes one contiguous 6KB run
    out_g = nc.declare_dram_parameter(
        "out_g", [NHT, 128, NWT, NR], OUT_DT, isOutput=True
    )

    with TileContext(nc) as tc:
        with (
            tc.tile_pool(name="bpool", bufs=1) as bpool,
            tc.tile_pool(name="apool", bufs=4) as apool,
            # 14 ht rows of tile-grams (84KB/partition) stage in SBUF, so
            # compute runs HAM-warm to the end (only the last 2 rows wait on
            # slot recycling, hidden under the drain).  HBM reads pipeline at
            # ~424GB/s on one ring but each HBM *write* pays a ~2us receipt
            # that serializes per ring, so the drain runs on TWO rings: even
            # ht rows on the otherwise-idle SWDGE ring, gated on a mid-tail
            # input chunk (the very last input packets straggle ~10us on slow
            # engines -- gating on the final chunk inherits that tail), odd
            # ht rows on the sync ring where the per-engine FIFO orders them
            # after the input stream naturally.  Scalar/vector carry ONLY
            # copies: a waiting DMA parked there head-of-line blocks them.
            tc.tile_pool(name="spool", bufs=14) as spool,
            tc.tile_pool(name="psum", bufs=6, space="PSUM") as ppool,
        ):
            # whole padded in2 sample resident in SBUF (72KB/partition),
            # loaded in 16-row chunks so early tiles can start before the
            # full 8.4MB lands (subtile deps give matmuls per-chunk waits)
            b_s = bpool.tile([128, KT, HP, WP], bf16)
            # zero the 4 pad rows top+bottom (pad cols ship from DRAM)
            nc.gpsimd.memset(b_s[:, :, 0:D, :], 0.0)
            nc.gpsimd.memset(b_s[:, :, D + H :, :], 0.0)

            def load_b(k):  # 16-row chunk k of the unpadded-row interior
                return nc.sync.dma_start(
                    out=b_s[:, :, D + 16 * k : D + 16 * k + 16],
                    in_=in2_p[:, :, 16 * k : 16 * k + 16],
                )

            # in1 rotates through 4 chunk buffers (2 ht rows each) on the
            # sync ring: with a 4-pair reuse margin the WAR waits are long
            # cleared by the time the sequencer reaches them, so the input
            # stream never parks
            a_tiles = []

            def load_a(t):
                a_t = apool.tile([128, 2, KT, NWT, 128], bf16, tag="a")
                a_tiles.append(a_t)
                return nc.sync.dma_start(out=a_t, in_=in1_t[:, 2 * t : 2 * t + 2])

            # all input DMAs issued up front (before any matmul reads, so no
            # conservative WAR hazards throttle the stream)
            gate = None
            for k in range(8):
                gb = load_b(k)
                ga = load_a(k)
                if k == 5:
                    gate = (gb, ga)

            st = None
            for ht in range(NHT):
                h0 = TH * ht
                for wt in range(NWT):
                    w0 = TW * wt
                    idx = ht * NWT + wt
                    ps = ppool.tile([128, NR], mybir.dt.float32, name="ps", tag="ps")
                    for kt in range(KT):
                        nc.tensor.matmul(
                            ps,
                            a_tiles[ht // 2][:, ht % 2, kt, wt, :],
                            b_s[:, kt, h0 : h0 + RH, w0 : w0 + RW],
                            start=(kt == 0),
                            stop=(kt == KT - 1),
                        )
                    if wt == 0:
                        st = spool.tile([128, NWT, NR], OUT_DT)
                    if idx % 2 == 0:
                        nc.vector.tensor_copy(st[:, wt, :], ps)
                    else:
                        nc.scalar.copy(st[:, wt, :], ps)
                    if wt == NWT - 1:
                        if ht % 2 == 1:
                            # sync ring: per-engine FIFO already orders this
                            # transfer after the input stream
                            nc.sync.dma_start(out=out_g[ht], in_=st)
                        else:
                            od = nc.gpsimd.dma_start(out=out_g[ht], in_=st)
                            add_dep_helper(od.ins, gate[0].ins, reason="drain near input end")
                            add_dep_helper(od.ins, gate[1].ins, reason="drain near input end")

    # bacc passes (move_matmul_waits_to_ldweights / generate_event_semaphores)
    # enforce the 1-wait-per-instruction HW constraint before serialization.
    nc.compile()
    return nc


def _get_nc():
    global _CACHED_NC
    if _CACHED_NC is None:
        _CACHED_NC = _build_nc()
    return _CACHED_NC


def _make_in_maps(in1: np.ndarray, in2: np.ndarray):
    in_maps = []
    for b in range(B):
        # [C,H,W] -> [c(128), ht, kt, wt, mh*16+mw]
        a = (
            in1[b]
            .astype(ml_dtypes.bfloat16)
            .reshape(KT, 128, NHT, TH, NWT, TW)
            .transpose(1, 2, 0, 4, 3, 5)
            .reshape(128, NHT, KT, NWT, 128)
        )
        p = np.zeros((KT, 128, H, WP), ml_dtypes.bfloat16)
        p[:, :, :, D : D + W] = in2[b].astype(ml_dtypes.bfloat16).reshape(
            KT, 128, H, W
        )
        in_maps.append(
            {
                "in1_t": np.ascontiguousarray(a),
                "in2_p": np.ascontiguousarray(p.transpose(1, 0, 2, 3)),
            }
        )
    return in_maps


_IH = np.arange(TH)[:, None]
_JW = np.arange(TW)[None, :]


def _extract_band(g: np.ndarray) -> np.ndarray:
    """[NHT, 128, NWT, NR] tile-grams -> [81, H, W] cost volume."""
    # -> [ht, wt, mh, mw, nh, nw]
    g2 = (
        g.reshape(NHT, 128, NWT, NR)
        .transpose(0, 2, 1, 3)
        .reshape(NHT, NWT, TH, TW, RH, RW)
        .astype(np.float32)
    )
    # windows over (nh, nw): sw[.., a, b, dy, dx] = g2[.., a+dy, b+dx]
    sw = np.lib.stride_tricks.sliding_window_view(g2, (ND, ND), axis=(4, 5))
    band = sw[:, :, _IH, _JW, _IH, _JW]  # [ht, wt, mh, mw, dy, dx]
    # -> [dy, dx, ht, mh, wt, mw] -> [81, H, W]
    return np.ascontiguousarray(band.transpose(4, 5, 0, 2, 1, 3)).reshape(ND * ND, H, W)


def kernel(**inputs) -> np.ndarray:
    in1 = np.ascontiguousarray(np.asarray(inputs["in1"], dtype=np.float32))
    in2 = np.ascontiguousarray(np.asarray(inputs["in2"], dtype=np.float32))
    assert in1.shape == (B, C, H, W) and in2.shape == (B, C, H, W)

    nc = _get_nc()
    in_maps = _make_in_maps(in1, in2)
    res = run_bass_kernel_spmd(nc, in_maps, list(range(B)))

    outs = [_extract_band(np.asarray(res.results[b]["out_g"])) for b in range(B)]
    return np.stack(outs).astype(np.float32)


# revision 30
# speedup vs baseline: 1.1673x; 1.0914x over previous
"""Correlation cost-volume kernel (max_displacement=4) for 8 Trainium2 cores.

Problem: in1, in2: [B=8, C=256, H=128, W=128] f32.
out[b, dy*9+dx, h, w] = sum_c in1[b,c,h,w] * pad(in2)[b, c, h+dy, w+dx]
(pad = 4 zeros on each spatial side), output [8, 81, 128, 128] f32.

Strategy (data-parallel, one batch sample per core):
  2D-tiled gram.  Each matmul tile covers an 8h x 16w block of in1 pixels
  (M = 128 PSUM partitions = pixels) against its 16 x 24 halo region of
  padded in2 (N = 384 columns, contracting C = 256 as two K=128 tiles
  accumulated in PSUM).  Every (pixel, displacement) pair the band needs is
  a (partition, column) entry of that [128 x 384] tile-gram, so the device
  streams 2x384 columns per 128 pixels -- 3.2x less TensorE work and 3.2x
  less output DMA than the per-row full-gram formulation.  The band entry
  for pixel (mh,mw) sits at column (mh+dy)*24 + (mw+dx): a per-partition
  (sheared) offset that no engine or DMA access pattern can express, so the
  device emits the full tile-grams (bf16, 4 tiles batched per DMA) and the
  host slices the 81-entry band per pixel with numpy stride tricks.
"""

import ml_dtypes
import numpy as np

import concourse.bass as bass
import concourse.bacc as bacc
import concourse.mybir as mybir
from concourse.bass_utils import run_bass_kernel_spmd
from concourse.tile import TileContext, add_dep_helper

B, C, H, W = 8, 256, 128, 128
D = 4
ND = 2 * D + 1  # 9 displacements per axis
HP = H + 2 * D  # 136 padded rows
WP = W + 2 * D  # 136 padded cols
KT = C // 128  # 2 contraction tiles
TH, TW = 8, 16  # matmul tile = 8h x 16w pixels (128 = one PSUM partition dim)
NHT, NWT = H // TH, W // TW  # 16 x 8 tiles
RH, RW = TH + 2 * D, TW + 2 * D  # 16 x 24 halo region, N = 384
NR = RH * RW  # 384 gram columns per tile

OUT_DT = mybir.dt.bfloat16
_OUT_NP = ml_dtypes.bfloat16

_CACHED_NC = None


def _build_nc():
    bf16 = mybir.dt.bfloat16

    nc = bacc.Bacc()
    # in1 as [c][ht][kt][wt][m=mh*16+mw]; in2 zero-padded in w only, as
    # [c][kt][h(128)][wp] -- the 8 zero pad rows are memset on-chip
    in1_t = nc.declare_dram_parameter("in1_t", [128, NHT, KT, NWT, 128], bf16, isOutput=False)
    in2_p = nc.declare_dram_parameter("in2_p", [128, KT, H, WP], bf16, isOutput=False)
    # tile-grams: [ht][m][wt][n]; partition m writes one contiguous 6KB run
    out_g = nc.declare_dram_parameter(
        "out_g", [NHT, 128, NWT, NR], OUT_DT, isOutput=True
    )

    with TileContext(nc) as tc:
        with (
            tc.tile_pool(name="bpool", bufs=1) as bpool,
            tc.tile_pool(name="apool", bufs=3) as apool,
            # 15 ht rows of tile-grams (90KB/partition) stage in SBUF, so
            # compute runs HAM-warm to the end.  HBM reads pipeline at
            # ~424GB/s on one ring but each HBM *write* pays a ~2us receipt
            # that serializes per ring, so the drain runs on multiple rings
            # -- but mostly HWDGE ones: heavy SWDGE traffic makes SDMA
            # engines 7/15 straggle ~10us (their AXI ports also serve the
            # SWDGE descriptor rings).  Scalar/vector queues carry copies
            # first: a waiting DMA parked ahead of copies blocks them.
            tc.tile_pool(name="spool", bufs=15) as spool,
            tc.tile_pool(name="psum", bufs=6, space="PSUM") as ppool,
        ):
            # whole padded in2 sample resident in SBUF (72KB/partition),
            # loaded in 16-row chunks so early tiles can start before the
            # full 8.4MB lands (subtile deps give matmuls per-chunk waits)
            b_s = bpool.tile([128, KT, HP, WP], bf16)
            # zero the 4 pad rows top+bottom (pad cols ship from DRAM)
            nc.gpsimd.memset(b_s[:, :, 0:D, :], 0.0)
            nc.gpsimd.memset(b_s[:, :, D + H :, :], 0.0)

            def load_b(k):  # 16-row chunk k of the unpadded-row interior
                return nc.sync.dma_start(
                    out=b_s[:, :, D + 16 * k : D + 16 * k + 16],
                    in_=in2_p[:, :, 16 * k : 16 * k + 16],
                )

            # in1 rotates through 3 chunk buffers (2 ht rows each) on the
            # sync ring: with a 3-pair reuse margin the WAR waits are long
            # cleared by the time the sequencer reaches them, so the input
            # stream never parks
            a_tiles = {}

            def load_a(t):
                a_t = apool.tile([128, 2, KT, NWT, 128], bf16, tag="a")
                a_tiles[t] = a_t
                return nc.sync.dma_start(out=a_t, in_=in1_t[:, 2 * t : 2 * t + 2])

            # all input DMAs issued up front (before any matmul reads, so no
            # conservative WAR hazards throttle the stream).  Chunk 7 loads
            # FIRST: ht row 15 is computed first, so its tile-gram drains
            # early (on the otherwise-idle SWDGE ring) and recycles the
            # staging slot the 16th row needs -- compute then never stalls
            # long enough to go HAM-cold.
            load_b(7)
            load_a(7)
            gate = None
            for k in range(7):
                gb = load_b(k)
                ga = load_a(k)
                if k == 5:
                    gate = (gb, ga)

            st = None
            scalar_outs = []
            for ht in [NHT - 1] + list(range(NHT - 1)):
                h0 = TH * ht
                for wt in range(NWT):
                    w0 = TW * wt
                    idx = ht * NWT + wt
                    ps = ppool.tile([128, NR], mybir.dt.float32, name="ps", tag="ps")
                    for kt in range(KT):
                        nc.tensor.matmul(
                            ps,
                            a_tiles[ht // 2][:, ht % 2, kt, wt, :],
                            b_s[:, kt, h0 : h0 + RH, w0 : w0 + RW],
                            start=(kt == 0),
                            stop=(kt == KT - 1),
                        )
                    if wt == 0:
                        st = spool.tile([128, NWT, NR], OUT_DT)
                    if idx % 2 == 0:
                        nc.vector.tensor_copy(st[:, wt, :], ps)
                    else:
                        nc.scalar.copy(st[:, wt, :], ps)
                    if wt == NWT - 1:
                        if ht == NHT - 1:
                            # lone SWDGE out (more SWDGE traffic slows SDMA
                            # engines 7/15 for everyone), gated near the
                            # input-stream end
                            od = nc.gpsimd.dma_start(out=out_g[ht], in_=st)
                            add_dep_helper(od.ins, gate[0].ins, reason="drain near input end")
                            add_dep_helper(od.ins, gate[1].ins, reason="drain near input end")
                        elif ht % 2 == 0:
                            # sync ring: per-engine FIFO already orders this
                            # transfer after the input stream
                            nc.sync.dma_start(out=out_g[ht], in_=st)
                        else:
                            # scalar-ring outs are emitted AFTER the loop so
                            # they cannot head-of-line block the copies
                            scalar_outs.append((ht, st))
            for ht, st_t in scalar_outs:
                nc.scalar.dma_start(out=out_g[ht], in_=st_t)

    # bacc passes (move_matmul_waits_to_ldweights / generate_event_semaphores)
    # enforce the 1-wait-per-instruction HW constraint before serialization.
    nc.compile()
    return nc


def _get_nc():
    global _CACHED_NC
    if _CACHED_NC is None:
        _CACHED_NC = _build_nc()
    return _CACHED_NC


def _make_in_maps(in1: np.ndarray, in2: np.ndarray):
    in_maps = []
    for b in range(B):
        # [C,H,W] -> [c(128), ht, kt, wt, mh*16+mw]
        a = (
            in1[b]
            .astype(ml_dtypes.bfloat16)
            .reshape(KT, 128, NHT, TH, NWT, TW)
            .transpose(1, 2, 0, 4, 3, 5)
            .reshape(128, NHT, KT, NWT, 128)
        )
        p = np.zeros((KT, 128, H, WP), ml_dtypes.bfloat16)
        p[:, :, :, D : D + W] = in2[b].astype(ml_dtypes.bfloat16).reshape(
            KT, 128, H, W
        )
        in_maps.append(
            {
                "in1_t": np.ascontiguousarray(a),
                "in2_p": np.ascontiguousarray(p.transpose(1, 0, 2, 3)),
            }
        )
    return in_maps


_IH = np.arange(TH)[:, None]
_JW = np.arange(TW)[None, :]


def _extract_band(g: np.ndarray) -> np.ndarray:
    """[NHT, 128, NWT, NR] tile-grams -> [81, H, W] cost volume."""
    # -> [ht, wt, mh, mw, nh, nw]
    g2 = (
        g.reshape(NHT, 128, NWT, NR)
        .transpose(0, 2, 1, 3)
        .reshape(NHT, NWT, TH, TW, RH, RW)
        .astype(np.float32)
    )
    # windows over (nh, nw): sw[.., a, b, dy, dx] = g2[.., a+dy, b+dx]
    sw = np.lib.stride_tricks.sliding_window_view(g2, (ND, ND), axis=(4, 5))
    band = sw[:, :, _IH, _JW, _IH, _JW]  # [ht, wt, mh, mw, dy, dx]
    # -> [dy, dx, ht, mh, wt, mw] -> [81, H, W]
    return np.ascontiguousarray(band.transpose(4, 5, 0, 2, 1, 3)).reshape(ND * ND, H, W)


def kernel(**inputs) -> np.ndarray:
    in1 = np.ascontiguousarray(np.asarray(inputs["in1"], dtype=np.float32))
    in2 = np.ascontiguousarray(np.asarray(inputs["in2"], dtype=np.float32))
    assert in1.shape == (B, C, H, W) and in2.shape == (B, C, H, W)

    nc = _get_nc()
    in_maps = _make_in_maps(in1, in2)
    res = run_bass_kernel_spmd(nc, in_maps, list(range(B)))

    outs = [_extract_band(np.asarray(res.results[b]["out_g"])) for b in range(B)]
    return np.stack(outs).astype(np.float32)


# revision 32
# speedup vs baseline: 1.2027x; 1.0304x over previous
"""Correlation cost-volume kernel (max_displacement=4) for 8 Trainium2 cores.

Problem: in1, in2: [B=8, C=256, H=128, W=128] f32.
out[b, dy*9+dx, h, w] = sum_c in1[b,c,h,w] * pad(in2)[b, c, h+dy, w+dx]
(pad = 4 zeros on each spatial side), output [8, 81, 128, 128] f32.

Strategy (data-parallel, one batch sample per core):
  2D-tiled gram.  Each matmul tile covers an 8h x 16w block of in1 pixels
  (M = 128 PSUM partitions = pixels) against its 16 x 24 halo region of
  padded in2 (N = 384 columns, contracting C = 256 as two K=128 tiles
  accumulated in PSUM).  Every (pixel, displacement) pair the band needs is
  a (partition, column) entry of that [128 x 384] tile-gram, so the device
  streams 2x384 columns per 128 pixels -- 3.2x less TensorE work and 3.2x
  less output DMA than the per-row full-gram formulation.  The band entry
  for pixel (mh,mw) sits at column (mh+dy)*24 + (mw+dx): a per-partition
  (sheared) offset that no engine or DMA access pattern can express, so the
  device emits the full tile-grams (bf16, 4 tiles batched per DMA) and the
  host slices the 81-entry band per pixel with numpy stride tricks.
"""

import ml_dtypes
import numpy as np

import concourse.bass as bass
import concourse.bacc as bacc
import concourse.mybir as mybir
from concourse.bass_utils import run_bass_kernel_spmd
from concourse.tile import TileContext, add_dep_helper

B, C, H, W = 8, 256, 128, 128
D = 4
ND = 2 * D + 1  # 9 displacements per axis
HP = H + 2 * D  # 136 padded rows
WP = W + 2 * D  # 136 padded cols
KT = C // 128  # 2 contraction tiles
TH, TW = 8, 16  # matmul tile = 8h x 16w pixels (128 = one PSUM partition dim)
NHT, NWT = H // TH, W // TW  # 16 x 8 tiles
RH, RW = TH + 2 * D, TW + 2 * D  # 16 x 24 halo region, N = 384
NR = RH * RW  # 384 gram columns per tile

OUT_DT = mybir.dt.bfloat16
_OUT_NP = ml_dtypes.bfloat16

_CACHED_NC = None


def _build_nc():
    bf16 = mybir.dt.bfloat16

    nc = bacc.Bacc()
    # in1 as [c][ht][kt][wt][m=mh*16+mw]; in2 zero-padded in w only, as
    # [c][kt][h(128)][wp] -- the 8 zero pad rows are memset on-chip
    in1_t = nc.declare_dram_parameter("in1_t", [128, NHT, KT, NWT, 128], bf16, isOutput=False)
    in2_p = nc.declare_dram_parameter("in2_p", [128, KT, H, WP], bf16, isOutput=False)
    # tile-grams: [ht][m][wt][n]; partition m writes one contiguous 6KB run
    out_g = nc.declare_dram_parameter(
        "out_g", [NHT, 128, NWT, NR], OUT_DT, isOutput=True
    )

    with TileContext(nc) as tc:
        with (
            tc.tile_pool(name="bpool", bufs=1) as bpool,
            tc.tile_pool(name="apool", bufs=3) as apool,
            # 15 ht rows of tile-grams (90KB/partition) stage in SBUF, so
            # compute runs HAM-warm to the end.  HBM reads pipeline at
            # ~424GB/s on one ring but each HBM *write* pays a ~2us receipt
            # that serializes per ring, so the drain runs on multiple rings
            # -- but mostly HWDGE ones: heavy SWDGE traffic makes SDMA
            # engines 7/15 straggle ~10us (their AXI ports also serve the
            # SWDGE descriptor rings).  Scalar/vector queues carry copies
            # first: a waiting DMA parked ahead of copies blocks them.
            tc.tile_pool(name="spool", bufs=15) as spool,
            tc.tile_pool(name="psum", bufs=6, space="PSUM") as ppool,
        ):
            # whole padded in2 sample resident in SBUF (72KB/partition),
            # loaded in 16-row chunks so early tiles can start before the
            # full 8.4MB lands (subtile deps give matmuls per-chunk waits)
            b_s = bpool.tile([128, KT, HP, WP], bf16)
            # zero the 4 pad rows top+bottom (pad cols ship from DRAM)
            nc.gpsimd.memset(b_s[:, :, 0:D, :], 0.0)
            nc.gpsimd.memset(b_s[:, :, D + H :, :], 0.0)

            def load_b(k):  # 16-row chunk k of the unpadded-row interior
                return nc.sync.dma_start(
                    out=b_s[:, :, D + 16 * k : D + 16 * k + 16],
                    in_=in2_p[:, :, 16 * k : 16 * k + 16],
                )

            # in1 rotates through 3 chunk buffers (2 ht rows each) on the
            # sync ring: with a 3-pair reuse margin the WAR waits are long
            # cleared by the time the sequencer reaches them, so the input
            # stream never parks
            a_tiles = {}

            def load_a(t):
                a_t = apool.tile([128, 2, KT, NWT, 128], bf16, tag="a")
                a_tiles[t] = a_t
                return nc.sync.dma_start(out=a_t, in_=in1_t[:, 2 * t : 2 * t + 2])

            # all input DMAs issued up front (before any matmul reads, so no
            # conservative WAR hazards throttle the stream).  Chunk 7 loads
            # FIRST: ht row 15 is computed first, so its tile-gram drains
            # early (on the otherwise-idle SWDGE ring) and recycles the
            # staging slot the 16th row needs -- compute then never stalls
            # long enough to go HAM-cold.
            load_b(7)
            load_a(7)
            for k in range(7):
                load_b(k)
                load_a(k)

            st = None
            for ht in [NHT - 1] + list(range(NHT - 1)):
                h0 = TH * ht
                for wt in range(NWT):
                    w0 = TW * wt
                    idx = ht * NWT + wt
                    ps = ppool.tile([128, NR], mybir.dt.float32, name="ps", tag="ps")
                    for kt in range(KT):
                        nc.tensor.matmul(
                            ps,
                            a_tiles[ht // 2][:, ht % 2, kt, wt, :],
                            b_s[:, kt, h0 : h0 + RH, w0 : w0 + RW],
                            start=(kt == 0),
                            stop=(kt == KT - 1),
                        )
                    if wt == 0:
                        st = spool.tile([128, NWT, NR], OUT_DT)
                    if idx % 2 == 0:
                        nc.vector.tensor_copy(st[:, wt, :], ps)
                    else:
                        nc.scalar.copy(st[:, wt, :], ps)
                    if wt == NWT - 1:
                        if ht % 2 == 0:
                            # sync ring: per-engine FIFO orders these after
                            # the input stream (pure write drain at the end)
                            nc.sync.dma_start(out=out_g[ht], in_=st)
                        else:
                            # scalar-HWDGE ring: drains each row as soon as
                            # its copies land (ht15 first, recycling the
                            # staging slot the 16th row needs).  Concurrent
                            # HWDGE read+write streams sustain ~427GB/s
                            # aggregate; no SWDGE -> engines 7/15 stay fast
                            nc.scalar.dma_start(out=out_g[ht], in_=st)

    # bacc passes (move_matmul_waits_to_ldweights / generate_event_semaphores)
    # enforce the 1-wait-per-instruction HW constraint before serialization.
    nc.compile()
    return nc


def _get_nc():
    global _CACHED_NC
    if _CACHED_NC is None:
        _CACHED_NC = _build_nc()
    return _CACHED_NC


def _make_in_maps(in1: np.ndarray, in2: np.ndarray):
    in_maps = []
    for b in range(B):
        # [C,H,W] -> [c(128), ht, kt, wt, mh*16+mw]
        a = (
            in1[b]
            .astype(ml_dtypes.bfloat16)
            .reshape(KT, 128, NHT, TH, NWT, TW)
            .transpose(1, 2, 0, 4, 3, 5)
            .reshape(128, NHT, KT, NWT, 128)
        )
        p = np.zeros((KT, 128, H, WP), ml_dtypes.bfloat16)
        p[:, :, :, D : D + W] = in2[b].astype(ml_dtypes.bfloat16).reshape(
            KT, 128, H, W
        )
        in_maps.append(
            {
                "in1_t": np.ascontiguousarray(a),
                "in2_p": np.ascontiguousarray(p.transpose(1, 0, 2, 3)),
            }
        )
    return in_maps


_IH = np.arange(TH)[:, None]
_JW = np.arange(TW)[None, :]


def _extract_band(g: np.ndarray) -> np.ndarray:
    """[NHT, 128, NWT, NR] tile-grams -> [81, H, W] cost volume."""
    # -> [ht, wt, mh, mw, nh, nw]
    g2 = (
        g.reshape(NHT, 128, NWT, NR)
        .transpose(0, 2, 1, 3)
        .reshape(NHT, NWT, TH, TW, RH, RW)
        .astype(np.float32)
    )
    # windows over (nh, nw): sw[.., a, b, dy, dx] = g2[.., a+dy, b+dx]
    sw = np.lib.stride_tricks.sliding_window_view(g2, (ND, ND), axis=(4, 5))
    band = sw[:, :, _IH, _JW, _IH, _JW]  # [ht, wt, mh, mw, dy, dx]
    # -> [dy, dx, ht, mh, wt, mw] -> [81, H, W]
    return np.ascontiguousarray(band.transpose(4, 5, 0, 2, 1, 3)).reshape(ND * ND, H, W)


def kernel(**inputs) -> np.ndarray:
    in1 = np.ascontiguousarray(np.asarray(inputs["in1"], dtype=np.float32))
    in2 = np.ascontiguousarray(np.asarray(inputs["in2"], dtype=np.float32))
    assert in1.shape == (B, C, H, W) and in2.shape == (B, C, H, W)

    nc = _get_nc()
    in_maps = _make_in_maps(in1, in2)
    res = run_bass_kernel_spmd(nc, in_maps, list(range(B)))

    outs = [_extract_band(np.asarray(res.results[b]["out_g"])) for b in range(B)]
    return np.stack(outs).astype(np.float32)
